# revision 2
# baseline (speedup 1.0000x reference)
"""Grouped MoE MLP (SwiGLU) for Trainium2, expert-parallel across 8 NeuronCores.

Problem: out = gmm(silu(gmm(x,Wg)) * gmm(x,Wu), Wd) with E=8 experts,
T=8192 tokens pre-sorted by expert, H=2048, I=4096.

Strategy: expert parallelism — core e computes expert e's tokens end-to-end.
The host splits the (ragged) token dim by expert, pads each group to a fixed
capacity C, casts everything to bf16 and relays weights out into the exact
tiled layouts the device program consumes, so every DMA line is contiguous.
Host relayouts run on a thread pool (they are memory-latency-bound gathers).

Device program per core (all shapes hardcoded at build time):
  GEMM1 computes the SwiGLU intermediate TRANSPOSED (interT[I, C]) so that
  GEMM2's contraction dim (I) is already the partition dim — no on-device
  transposes anywhere. bf16 inputs, fp32 PSUM accumulation, bf16 output.
  Weight streams alternate between the two HWDGE rings (sync / scalar) so
  the per-ring ~0.6us issue cost doesn't serialize the startup ramp.
"""

import numpy as np
import ml_dtypes
from concurrent.futures import ThreadPoolExecutor

P = 128          # partition dim
NB = 512         # matmul moving free dim / PSUM bank width (fp32)
E, T, H, I = 8, 8192, 2048, 4096
C_DEFAULT = T // E  # per-expert token capacity

_NC_CACHE = {}


def _build(C, Hd, Id, nb=NB):
    """Build + bacc-compile the per-core Tile program. Returns the Bass module."""
    import concourse.bass as bass  # noqa: F401
    import concourse.tile as tile
    from concourse import bacc, mybir

    bf16 = mybir.dt.bfloat16
    f32 = mybir.dt.float32
    KT = Hd // P       # GEMM1 contraction tiles (over H)
    IT = Id // P       # i-tiles (GEMM1 output partitions / GEMM2 contraction)
    TT = C // nb       # token blocks for GEMM1 moving operand
    T8 = C // P        # token tiles for GEMM2 output partitions
    HB = Hd // nb      # h-blocks for GEMM2 moving operand

    nc = bacc.Bacc(
        "TRN2",
        target_bir_lowering=False,
        debug=False,
        enable_asserts=False,
        num_devices=8,
    )
    xT = nc.dram_tensor("xT", [Hd, C], bf16, kind="ExternalInput").ap()
    wg = nc.dram_tensor("wg", [IT, P, Hd], bf16, kind="ExternalInput").ap()
    wu = nc.dram_tensor("wu", [IT, P, Hd], bf16, kind="ExternalInput").ap()
    wd = nc.dram_tensor("wd", [HB, P, IT * nb], bf16, kind="ExternalInput").ap()
    out = nc.dram_tensor("out", [C, Hd], bf16, kind="ExternalOutput").ap()

    DSPL = 4  # split big wd-block DMAs across queues
    with tile.TileContext(nc) as tc:
        with tc.tile_pool(name="res", bufs=1) as res:
            # SwiGLU intermediate, transposed: interT[p, i*C + c] = inter[c, i*P+p]
            interT = res.tile([P, IT * C], bf16)
            # h=0 block of Wd, prefetched during phase 1 so phase 2 starts hot
            wd0 = res.tile([P, IT * nb], bf16)

            # ---------------- Phase 1: gate/up GEMMs + SwiGLU ----------------
            with tc.tile_pool(name="p1x", bufs=1) as p1x, \
                 tc.tile_pool(name="w1", bufs=3) as w1, \
                 tc.tile_pool(name="ps1", bufs=2, space="PSUM") as ps1, \
                 tc.tile_pool(name="tmp1", bufs=4) as tmp1:
            # Startup critical path: the very first psum group needs wg[0]'s
            # k=0 slice plus xt's (t=0, k) chunks.  Chunk the i=0 weight
            # loads and spread everything across both HWDGE rings so the
            # first matmul fires as early as possible and the xt ramp
            # isn't serialized behind one ring's ~0.6us-per-DMA issue cost.
                wgi0 = w1.tile([P, Hd], bf16, tag="wg")
                wui0 = w1.tile([P, Hd], bf16, tag="wu")
                qw = Hd // 4
                nc.sync.dma_start(wgi0[:, 0:qw], wg[0][:, 0:qw])
                nc.scalar.dma_start(wui0[:, 0:qw], wu[0][:, 0:qw])
                # xt[p, k*C + c] = x[c, k*P+p]  (resident, 32KB/partition);
                # t=0 halves first so the first psum group can start sooner
                xt = p1x.tile([P, KT * C], bf16)
                for t0 in range(0, C, nb):
                    for k in range(KT):
                        eng = nc.sync if k % 2 == 0 else nc.scalar
                        eng.dma_start(xt[:, k * C + t0: k * C + t0 + nb],
                                      xT[k * P:(k + 1) * P, t0:t0 + nb])
                for d in range(1, 4):
                    nc.sync.dma_start(wgi0[:, d * qw:(d + 1) * qw],
                                      wg[0][:, d * qw:(d + 1) * qw])
                    nc.scalar.dma_start(wui0[:, d * qw:(d + 1) * qw],
                                        wu[0][:, d * qw:(d + 1) * qw])
                for i in range(IT):
                    if i == 0:
                        wgi, wui = wgi0, wui0
                    else:
                        wgi = w1.tile([P, Hd], bf16, tag="wg")
                        nc.sync.dma_start(wgi[:], wg[i])
                        wui = w1.tile([P, Hd], bf16, tag="wu")
                        nc.scalar.dma_start(wui[:], wu[i])
                        if i == 8:
                            # prefetch Wd h=0 once the startup ramp (xt +
                            # first weight tiles) has drained; phase 2 only
                            # needs it at ~2/3 of the kernel span
                            for d in range(DSPL):
                                w = IT * nb // DSPL
                                eng = nc.sync if d % 2 == 0 else nc.scalar
                                eng.dma_start(wd0[:, d * w:(d + 1) * w],
                                              wd[0][:, d * w:(d + 1) * w])
                    for t in range(TT):
                        psg = ps1.tile([P, nb], f32, tag=f"g{t}")
                        psu = ps1.tile([P, nb], f32, tag=f"u{t}")
                        for k in range(KT):
                            rhs = xt[:, k * C + t * nb: k * C + t * nb + nb]
                            nc.tensor.matmul(psg[:], wgi[:, k * P:(k + 1) * P], rhs,
                                             start=(k == 0), stop=(k == KT - 1))
                        for k in range(KT):
                            rhs = xt[:, k * C + t * nb: k * C + t * nb + nb]
                            nc.tensor.matmul(psu[:], wui[:, k * P:(k + 1) * P], rhs,
                                             start=(k == 0), stop=(k == KT - 1))
                        # silu(g)*u = sigmoid(g)*g*u  (Silu LUT not in CoreSim);
                        # each DVE op may read at most ONE operand from PSUM.
                        sig = tmp1.tile([P, nb], f32, tag="sig")
                        nc.scalar.activation(sig[:], psg[:], mybir.ActivationFunctionType.Sigmoid)
                        sg = tmp1.tile([P, nb], f32, tag="sg")
                        nc.vector.tensor_mul(sg[:], sig[:], psg[:])
                        nc.vector.tensor_mul(
                            interT[:, i * C + t * nb: i * C + t * nb + nb], sg[:], psu[:])

            # ---------------- Phase 2: down GEMM ----------------
            with tc.tile_pool(name="w2", bufs=2) as w2, \
                 tc.tile_pool(name="ps2", bufs=4, space="PSUM") as ps2, \
                 tc.tile_pool(name="ot2", bufs=4) as ot2:
                for h in range(HB):
                    if h == 0:
                        wdh = wd0
                    else:
                        wdh = w2.tile([P, IT * nb], bf16, tag="wd")
                        for d in range(DSPL):
                            w = IT * nb // DSPL
                            eng = nc.sync if d % 2 == 0 else nc.scalar
                            eng.dma_start(wdh[:, d * w:(d + 1) * w],
                                          wd[h][:, d * w:(d + 1) * w])
                    for t in range(T8):
                        ps = ps2.tile([P, nb], f32, tag="o")
                        for k in range(IT):
                            nc.tensor.matmul(
                                ps[:],
                                interT[:, k * C + t * P: k * C + t * P + P],
                                wdh[:, k * nb:(k + 1) * nb],
                                start=(k == 0), stop=(k == IT - 1))
                        ot = ot2.tile([P, nb], bf16, tag="ot")
                        nc.scalar.copy(ot[:], ps[:])
                        nc.sync.dma_start(out[t * P:(t + 1) * P, h * nb:(h + 1) * nb], ot[:])

    nc.compile()
    return nc


def _get_nc(C, Hd, Id):
    key = (C, Hd, Id)
    if key not in _NC_CACHE:
        _NC_CACHE[key] = _build(C, Hd, Id)
    return _NC_CACHE[key]


def _prepare(inputs):
    """Host-side dispatch: split tokens by expert, pad to capacity, cast to
    bf16, relayout weights into the device's tiled DRAM formats.  The 32
    independent cast+relayout jobs run on a thread pool — they are
    cache-cold strided gathers, so threads overlap the memory latency."""
    bf = ml_dtypes.bfloat16
    x = np.asarray(inputs["permuted_local_hidden_states"], dtype=np.float32)
    tpe = np.asarray(inputs["tokens_per_expert"], dtype=np.int64)
    gate = np.asarray(inputs["gate_proj"], dtype=np.float32)
    up = np.asarray(inputs["up_proj"], dtype=np.float32)
    down = np.asarray(inputs["down_proj"], dtype=np.float32)

    Ee, Hd, Id = gate.shape
    Tt = x.shape[0]
    assert Ee == E, f"expected {E} experts, got {Ee}"
    counts = [int(c) for c in tpe]
    starts = [0]
    for c in counts:
        starts.append(starts[-1] + c)
    cmax = max(max(counts), 1)
    C = max(C_DEFAULT, ((cmax + P - 1) // P) * P)

    KT, IT, HB = Hd // P, Id // P, Hd // NB

    def _x_job(e):
        s, cnt = starts[e], counts[e]
        if cnt == C:
            xe = x[s:s + cnt]
        else:
            xe = np.zeros((C, Hd), np.float32)
            xe[:cnt] = x[s:s + cnt]
        return np.ascontiguousarray(xe.T).astype(bf)

    def _wg_job(e):
        return np.ascontiguousarray(
            gate[e].astype(bf).reshape(KT, P, IT, P).transpose(2, 1, 0, 3)
        ).reshape(IT, P, Hd)

    def _wu_job(e):
        return np.ascontiguousarray(
            up[e].astype(bf).reshape(KT, P, IT, P).transpose(2, 1, 0, 3)
        ).reshape(IT, P, Hd)

    def _wd_job(e):
        return np.ascontiguousarray(
            down[e].astype(bf).reshape(IT, P, HB, NB).transpose(2, 1, 0, 3)
        ).reshape(HB, P, IT * NB)

    jobs = [(kind, e) for e in range(Ee) for kind in ("xT", "wg", "wu", "wd")]
    fns = {"xT": _x_job, "wg": _wg_job, "wu": _wu_job, "wd": _wd_job}
    with ThreadPoolExecutor(max_workers=16) as pool:
        futs = {(kind, e): pool.submit(fns[kind], e) for kind, e in jobs}
        in_maps = [
            {kind: futs[(kind, e)].result() for kind in ("xT", "wg", "wu", "wd")}
            for e in range(Ee)
        ]
    meta = (Tt, Hd, starts, counts, C)
    return in_maps, meta


def _postprocess(results, meta):
    Tt, Hd, starts, counts, _C = meta
    outf = np.zeros((Tt, Hd), np.float32)
    for e in range(len(counts)):
        s, cnt = starts[e], counts[e]
        if cnt > 0:
            outf[s:s + cnt] = np.asarray(results[e]["out"])[:cnt].astype(np.float32)
    return outf


def kernel(**inputs):
    from concourse.bass_utils import run_bass_kernel_spmd
    in_maps, meta = _prepare(inputs)
    nc = _get_nc(meta[4], meta[1], np.asarray(inputs["gate_proj"]).shape[2])
    res = run_bass_kernel_spmd(nc, in_maps, list(range(E)))
    return _postprocess(res.results, meta)


# revision 4
# speedup vs baseline: 1.0101x; 1.0101x over previous
"""Grouped MoE MLP (SwiGLU) for Trainium2, expert-parallel across 8 NeuronCores.

Problem: out = gmm(silu(gmm(x,Wg)) * gmm(x,Wu), Wd) with E=8 experts,
T=8192 tokens pre-sorted by expert, H=2048, I=4096.

Strategy: expert parallelism — core e computes expert e's tokens end-to-end.
The host splits the (ragged) token dim by expert, pads each group to a fixed
capacity C, casts everything to bf16 and relays weights out into the exact
tiled layouts the device program consumes, so every DMA line is contiguous.
Host relayouts run on a thread pool (they are memory-latency-bound gathers).

Device program per core (all shapes hardcoded at build time):
  GEMM1 computes the SwiGLU intermediate TRANSPOSED (interT[I, C]) so that
  GEMM2's contraction dim (I) is already the partition dim — no on-device
  transposes anywhere. bf16 inputs, fp32 PSUM accumulation, bf16 output.
  Weight streams alternate between the two HWDGE rings (sync / scalar) so
  the per-ring ~0.6us issue cost doesn't serialize the startup ramp.
"""

import numpy as np
import ml_dtypes
from concurrent.futures import ThreadPoolExecutor

P = 128          # partition dim
NB = 512         # matmul moving free dim / PSUM bank width (fp32)
E, T, H, I = 8, 8192, 2048, 4096
C_DEFAULT = T // E  # per-expert token capacity

_NC_CACHE = {}


def _build(C, Hd, Id, nb=NB):
    """Build + bacc-compile the per-core Tile program. Returns the Bass module."""
    import concourse.bass as bass  # noqa: F401
    import concourse.tile as tile
    from concourse import bacc, mybir

    bf16 = mybir.dt.bfloat16
    f32 = mybir.dt.float32
    KT = Hd // P       # GEMM1 contraction tiles (over H)
    IT = Id // P       # i-tiles (GEMM1 output partitions / GEMM2 contraction)
    TT = C // nb       # token blocks for GEMM1 moving operand
    T8 = C // P        # token tiles for GEMM2 output partitions
    HB = Hd // nb      # h-blocks for GEMM2 moving operand

    nc = bacc.Bacc(
        "TRN2",
        target_bir_lowering=False,
        debug=False,
        enable_asserts=False,
        num_devices=8,
    )
    xT = nc.dram_tensor("xT", [Hd, C], bf16, kind="ExternalInput").ap()
    wg = nc.dram_tensor("wg", [IT, P, Hd], bf16, kind="ExternalInput").ap()
    wu = nc.dram_tensor("wu", [IT, P, Hd], bf16, kind="ExternalInput").ap()
    wd = nc.dram_tensor("wd", [HB, P, IT * nb], bf16, kind="ExternalInput").ap()
    out = nc.dram_tensor("out", [C, Hd], bf16, kind="ExternalOutput").ap()

    DSPL = 4  # split big wd-block DMAs across queues
    with tile.TileContext(nc) as tc:
        with tc.tile_pool(name="res", bufs=1) as res:
            # SwiGLU intermediate, transposed: interT[p, i*C + c] = inter[c, i*P+p]
            interT = res.tile([P, IT * C], bf16)
            # h=0 block of Wd, prefetched during phase 1 so phase 2 starts hot
            wd0 = res.tile([P, IT * nb], bf16)

            # ---------------- Phase 1: gate/up GEMMs + SwiGLU ----------------
            with tc.tile_pool(name="p1x", bufs=1) as p1x, \
                 tc.tile_pool(name="w1", bufs=3) as w1, \
                 tc.tile_pool(name="ps1", bufs=2, space="PSUM") as ps1, \
                 tc.tile_pool(name="tmp1", bufs=4) as tmp1:
            # Startup critical path: the first psum group consumes wg[0]'s
            # k=0 slice plus xt chunks k=0..KT-1 in order.  HWDGE issue
            # costs ~0.6us per dma_start per ring, so the ramp is bound by
            # ISSUE-slot serialization, not HBM bandwidth: use few, large
            # xt DMAs (one per k, arriving in the exact order the first
            # matmul group consumes them) and interleave them with the
            # weight stream across both rings (wg on sync, wu on scalar).
                wgi0 = w1.tile([P, Hd], bf16, tag="wg")
                wui0 = w1.tile([P, Hd], bf16, tag="wu")
                qw = Hd // 4
                xt = p1x.tile([P, KT * C], bf16)
                nc.sync.dma_start(wgi0[:, 0:qw], wg[0][:, 0:qw])
                nc.scalar.dma_start(wui0[:, 0:qw], wu[0][:, 0:qw])
                # xt[p, k*C + c] = x[c, k*P+p]  (resident, 32KB/partition)
                for k in range(KT):
                    eng = nc.sync if k % 2 == 0 else nc.scalar
                    eng.dma_start(xt[:, k * C:(k + 1) * C], xT[k * P:(k + 1) * P, :])
                    if k < 3:
                        nc.sync.dma_start(wgi0[:, (k + 1) * qw:(k + 2) * qw],
                                          wg[0][:, (k + 1) * qw:(k + 2) * qw])
                        nc.scalar.dma_start(wui0[:, (k + 1) * qw:(k + 2) * qw],
                                            wu[0][:, (k + 1) * qw:(k + 2) * qw])
                for i in range(IT):
                    if i == 0:
                        wgi, wui = wgi0, wui0
                    else:
                        wgi = w1.tile([P, Hd], bf16, tag="wg")
                        nc.sync.dma_start(wgi[:], wg[i])
                        wui = w1.tile([P, Hd], bf16, tag="wu")
                        nc.scalar.dma_start(wui[:], wu[i])
                        if i == 8:
                            # prefetch Wd h=0 once the startup ramp (xt +
                            # first weight tiles) has drained; phase 2 only
                            # needs it at ~2/3 of the kernel span
                            for d in range(DSPL):
                                w = IT * nb // DSPL
                                eng = nc.sync if d % 2 == 0 else nc.scalar
                                eng.dma_start(wd0[:, d * w:(d + 1) * w],
                                              wd[0][:, d * w:(d + 1) * w])
                    for t in range(TT):
                        psg = ps1.tile([P, nb], f32, tag=f"g{t}")
                        psu = ps1.tile([P, nb], f32, tag=f"u{t}")
                        for k in range(KT):
                            rhs = xt[:, k * C + t * nb: k * C + t * nb + nb]
                            nc.tensor.matmul(psg[:], wgi[:, k * P:(k + 1) * P], rhs,
                                             start=(k == 0), stop=(k == KT - 1))
                        for k in range(KT):
                            rhs = xt[:, k * C + t * nb: k * C + t * nb + nb]
                            nc.tensor.matmul(psu[:], wui[:, k * P:(k + 1) * P], rhs,
                                             start=(k == 0), stop=(k == KT - 1))
                        # silu(g)*u = sigmoid(g)*g*u  (Silu LUT not in CoreSim);
                        # each DVE op may read at most ONE operand from PSUM.
                        sig = tmp1.tile([P, nb], f32, tag="sig")
                        nc.scalar.activation(sig[:], psg[:], mybir.ActivationFunctionType.Sigmoid)
                        sg = tmp1.tile([P, nb], f32, tag="sg")
                        nc.vector.tensor_mul(sg[:], sig[:], psg[:])
                        nc.vector.tensor_mul(
                            interT[:, i * C + t * nb: i * C + t * nb + nb], sg[:], psu[:])

            # ---------------- Phase 2: down GEMM ----------------
            with tc.tile_pool(name="w2", bufs=2) as w2, \
                 tc.tile_pool(name="ps2", bufs=4, space="PSUM") as ps2, \
                 tc.tile_pool(name="ot2", bufs=4) as ot2:
                for h in range(HB):
                    if h == 0:
                        wdh = wd0
                    else:
                        wdh = w2.tile([P, IT * nb], bf16, tag="wd")
                        for d in range(DSPL):
                            w = IT * nb // DSPL
                            eng = nc.sync if d % 2 == 0 else nc.scalar
                            eng.dma_start(wdh[:, d * w:(d + 1) * w],
                                          wd[h][:, d * w:(d + 1) * w])
                    for t in range(T8):
                        ps = ps2.tile([P, nb], f32, tag="o")
                        for k in range(IT):
                            nc.tensor.matmul(
                                ps[:],
                                interT[:, k * C + t * P: k * C + t * P + P],
                                wdh[:, k * nb:(k + 1) * nb],
                                start=(k == 0), stop=(k == IT - 1))
                        ot = ot2.tile([P, nb], bf16, tag="ot")
                        nc.scalar.copy(ot[:], ps[:])
                        nc.sync.dma_start(out[t * P:(t + 1) * P, h * nb:(h + 1) * nb], ot[:])

    nc.compile()
    return nc


def _get_nc(C, Hd, Id):
    key = (C, Hd, Id)
    if key not in _NC_CACHE:
        _NC_CACHE[key] = _build(C, Hd, Id)
    return _NC_CACHE[key]


def _prepare(inputs):
    """Host-side dispatch: split tokens by expert, pad to capacity, cast to
    bf16, relayout weights into the device's tiled DRAM formats.  The 32
    independent cast+relayout jobs run on a thread pool — they are
    cache-cold strided gathers, so threads overlap the memory latency."""
    bf = ml_dtypes.bfloat16
    x = np.asarray(inputs["permuted_local_hidden_states"], dtype=np.float32)
    tpe = np.asarray(inputs["tokens_per_expert"], dtype=np.int64)
    gate = np.asarray(inputs["gate_proj"], dtype=np.float32)
    up = np.asarray(inputs["up_proj"], dtype=np.float32)
    down = np.asarray(inputs["down_proj"], dtype=np.float32)

    Ee, Hd, Id = gate.shape
    Tt = x.shape[0]
    assert Ee == E, f"expected {E} experts, got {Ee}"
    counts = [int(c) for c in tpe]
    starts = [0]
    for c in counts:
        starts.append(starts[-1] + c)
    cmax = max(max(counts), 1)
    C = max(C_DEFAULT, ((cmax + P - 1) // P) * P)

    KT, IT, HB = Hd // P, Id // P, Hd // NB

    # The strided relayout copies run on uint16/float32 views — native
    # dtypes release the GIL inside numpy's copy loops, so the pool
    # actually parallelizes (ml_dtypes' bfloat16 copy loops do not).
    def _x_job(e):
        s, cnt = starts[e], counts[e]
        if cnt == C:
            xe = x[s:s + cnt]
        else:
            xe = np.zeros((C, Hd), np.float32)
            xe[:cnt] = x[s:s + cnt]
        return np.ascontiguousarray(xe.T).astype(bf)

    def _w1_job(w, e):
        a = w[e].astype(bf).view(np.uint16)
        r = np.ascontiguousarray(a.reshape(KT, P, IT, P).transpose(2, 1, 0, 3))
        return r.reshape(IT, P, Hd).view(bf)

    def _wg_job(e):
        return _w1_job(gate, e)

    def _wu_job(e):
        return _w1_job(up, e)

    def _wd_job(e):
        a = down[e].astype(bf).view(np.uint16)
        r = np.ascontiguousarray(a.reshape(IT, P, HB, NB).transpose(2, 1, 0, 3))
        return r.reshape(HB, P, IT * NB).view(bf)

    jobs = [(kind, e) for e in range(Ee) for kind in ("xT", "wg", "wu", "wd")]
    fns = {"xT": _x_job, "wg": _wg_job, "wu": _wu_job, "wd": _wd_job}
    with ThreadPoolExecutor(max_workers=16) as pool:
        futs = {(kind, e): pool.submit(fns[kind], e) for kind, e in jobs}
        in_maps = [
            {kind: futs[(kind, e)].result() for kind in ("xT", "wg", "wu", "wd")}
            for e in range(Ee)
        ]
    meta = (Tt, Hd, starts, counts, C)
    return in_maps, meta


def _postprocess(results, meta):
    Tt, Hd, starts, counts, _C = meta
    outf = np.zeros((Tt, Hd), np.float32)
    for e in range(len(counts)):
        s, cnt = starts[e], counts[e]
        if cnt > 0:
            outf[s:s + cnt] = np.asarray(results[e]["out"])[:cnt].astype(np.float32)
    return outf


def kernel(**inputs):
    from concourse.bass_utils import run_bass_kernel_spmd
    in_maps, meta = _prepare(inputs)
    nc = _get_nc(meta[4], meta[1], np.asarray(inputs["gate_proj"]).shape[2])
    res = run_bass_kernel_spmd(nc, in_maps, list(range(E)))
    return _postprocess(res.results, meta)
